# revision 4
# baseline (speedup 1.0000x reference)
import sys
import numpy as np

for _p in ("/opt/trn_rl_repo", "/root/.axon_site/_ro/trn_rl_repo"):
    if _p not in sys.path:
        sys.path.append(_p)

B, N, NODE, FE = 128, 100, 2, 128
NODE_SIZES = [2, 16, 32]
FN2_OUT = [14, 30, 1]
NCORES = 8
GPC = B // NCORES            # graphs per core = 16
COLS = GPC * N               # 1600
PAIRS = N * N                # 10000
CH = 500                     # matmul moving chunk (<=512 fp32)
IPG = [5, 5, 5, 5, 5, 5, 5, 5, 5, 5, 5, 5, 5, 5, 5, 5, 5, 5, 5, 5]  # i's per chunk


def round_fp32r(a):
    u = np.ascontiguousarray(np.asarray(a, np.float32)).view(np.uint32)
    low = u & np.uint32(0xFFF)
    base = u & np.uint32(0xFFFFF000)
    add = ((low > 0x800) | ((low == 0x800) & (((u >> 12) & 1) == 1))).astype(np.uint32) << 12
    return (base + add).view(np.float32)


_CACHE = {}


def _build():
    import concourse.bacc as bacc
    import concourse.mybir as mybir
    import concourse.tile as tile

    F32 = mybir.dt.float32
    F32R = mybir.dt.float32r
    AF = mybir.ActivationFunctionType
    ALU = mybir.AluOpType
    AX = mybir.AxisListType

    nc = bacc.Bacc("TRN2", target_bir_lowering=False, debug=False, num_devices=NCORES)

    din = {}

    def I(name, shape, dt=F32R):
        din[name] = nc.dram_tensor(name, shape, dt, kind="ExternalInput")

    I("xt0", [2, COLS])
    I("m2xt", [2, COLS])
    I("sqv", [1, COLS])
    I("onesv", [1, COLS])
    for i in range(3):
        d = NODE_SIZES[i]
        od = FN2_OUT[i]
        I(f"w1a{i}", [d, 128])
        I(f"w1b{i}", [d, 128])
        I(f"w1c{i}", [1, 128])
        I(f"fe2T{i}", [128, 128])
        I(f"fn1avT{i}", [128, 128], F32)
        I(f"fn1xT{i}", [d, 128])
        I(f"fn2T{i}", [128, od], F32)
        I(f"b1_{i}", [128, 1], F32)
        I(f"b2_{i}", [128, 1], F32)
        I(f"fb1_{i}", [128, 1], F32)
        I(f"fb2_{i}", [od, 1], F32)
    out_d = nc.dram_tensor("out", [1, GPC], F32, kind="ExternalOutput")
    nrm_dram = nc.dram_tensor("nrm_dram", [GPC, PAIRS], F32R)

    # chunk grouping: groups of 3 chunks (1500 cols) -> PSUM tile 3 banks
    GRP_CH = [(0, 3), (3, 3), (6, 3), (9, 3), (12, 3), (15, 3), (18, 2)]

    with tile.TileContext(nc) as tc:
        with (
            tc.tile_pool(name="const", bufs=1) as cpool,
            tc.tile_pool(name="xp", bufs=1) as xpool,
            tc.tile_pool(name="wk", bufs=3) as wpool,
            tc.tile_pool(name="wk2", bufs=2) as wpool2,
            tc.tile_pool(name="stg", bufs=2) as spool,
            tc.tile_pool(name="ps1", bufs=1, space="PSUM") as ps1,
            tc.tile_pool(name="ps2", bufs=1, space="PSUM") as ps2,
            tc.tile_pool(name="psm", bufs=1, space="PSUM") as psm,
        ):
            W = {}
            for name, dt_ in din.items():
                sh = list(dt_.shape)
                W[name] = cpool.tile(sh, dt_.dtype, tag=name, name=f"w_{name}")
                nc.sync.dma_start(W[name][:], dt_.ap())

            xt0 = W["xt0"]

            # ---------- phase 0: per-graph pair norms -> nrm_dram ----------
            for g in range(GPC):
                sl = slice(g * N, (g + 1) * N)
                pn = psm.tile([100, 100], F32, tag="psm")
                nc.tensor.matmul(pn[:], xt0[:, sl], W["m2xt"][:, sl], start=True, stop=False)
                nc.tensor.matmul(pn[:], W["sqv"][:, sl], W["onesv"][:, sl], start=False, stop=False)
                nc.tensor.matmul(pn[:], W["onesv"][:, sl], W["sqv"][:, sl], start=False, stop=True)
                ncl = wpool.tile([100, 100], F32, tag="nclamp")
                nc.vector.tensor_scalar(ncl[:], pn[:], 0.0, None, ALU.max)
                nsq = wpool.tile([100, 100], F32R, tag="nsq")
                nc.scalar.activation(nsq[:], ncl[:], AF.Sqrt)
                nc.sync.dma_start(
                    nrm_dram[g:g + 1, :].rearrange("p (a b) -> p a b", a=100), nsq[:, :]
                )

            # ---------- blocks ----------
            xt = xt0
            ty = None
            for blk in range(3):
                d = NODE_SIZES[blk]
                od = FN2_OUT[blk]
                w1a, w1b, w1c = W[f"w1a{blk}"], W[f"w1b{blk}"], W[f"w1c{blk}"]
                fe2T = W[f"fe2T{blk}"]
                b1, b2 = W[f"b1_{blk}"], W[f"b2_{blk}"]
                AV = xpool.tile([128, COLS], F32, tag=f"av{blk}")

                for g in range(GPC):
                    gsl = slice(g * N, (g + 1) * N)
                    stg = spool.tile([1, PAIRS], F32R, tag="stg")
                    nc.sync.dma_start(stg[:], nrm_dram[g:g + 1, :])

                    for (c0, nch) in GRP_CH:
                        span = nch * CH
                        p1 = ps1.tile([128, 1536], F32, tag="p1")
                        # fe1 = x1 part + x2 part + norm part (accumulate in PSUM)
                        # chunk ci lives at bank-aligned offset ci*512, cols [0,500)
                        for ci in range(nch):
                            i0 = (c0 + ci) * 5
                            r1 = xt[:, g * N + i0: g * N + i0 + 5].unsqueeze(2).broadcast_to([d, 5, N])
                            nc.tensor.matmul(p1[:, ci * 512: ci * 512 + CH], w1a[:], r1,
                                             start=True, stop=False)
                        for ci in range(nch):
                            r2 = xt[:, gsl].unsqueeze(1).broadcast_to([d, 5, N])
                            nc.tensor.matmul(p1[:, ci * 512: ci * 512 + CH], w1b[:], r2,
                                             start=False, stop=False)
                        for ci in range(nch):
                            cc = (c0 + ci) * CH
                            nc.tensor.matmul(p1[:, ci * 512: ci * 512 + CH], w1c[:],
                                             stg[:, cc:cc + CH], start=False, stop=True)
                        # Lrelu (exact, alpha=0.2, bias=b1) -> t1
                        p1v = p1[:, 0:nch * 512].rearrange("p (a b) -> p a b", b=512)[:, :, 0:CH]
                        t1 = wpool.tile([128, 1500], F32R, tag="t1")
                        t1v = t1[:, 0:span].rearrange("p (a b) -> p a b", b=CH)
                        nc.scalar.activation(t1v, p1v, AF.Prelu,
                                             bias=b1[:], scale=1.0, alpha=0.2)
                        # fe2
                        p2 = ps2.tile([128, 1536], F32, tag="p2")
                        for ci in range(nch):
                            nc.tensor.matmul(p2[:, ci * 512: ci * 512 + CH], fe2T[:],
                                             t1[:, ci * CH:(ci + 1) * CH], start=True, stop=True)
                        p2v = p2[:, 0:nch * 512].rearrange("p (a b) -> p a b", b=512)[:, :, 0:CH]
                        av2 = wpool2.tile([128, 1500], F32, tag="av2")
                        av2v = av2[:, 0:span].rearrange("p (a b) -> p a b", b=CH)
                        nc.scalar.activation(av2v, p2v, AF.Prelu,
                                             bias=b2[:], scale=1.0, alpha=0.2)
                        # sum over j within each i (segments of 100)
                        n_i = span // N
                        nc.vector.tensor_reduce(
                            AV[:, g * N + c0 * 5: g * N + c0 * 5 + n_i],
                            av2[:, 0:span].rearrange("p (a b) -> p a b", a=n_i),
                            axis=AX.X, op=ALU.add,
                        )

                # ----- fn stage over all COLS -----
                fn1avT, fn1xT = W[f"fn1avT{blk}"], W[f"fn1xT{blk}"]
                fn2T = W[f"fn2T{blk}"]
                fb1, fb2 = W[f"fb1_{blk}"], W[f"fb2_{blk}"]
                if blk < 2:
                    nd = NODE_SIZES[blk + 1]
                    xnext = xpool.tile([nd, COLS], F32R, tag=f"x{blk + 1}")
                else:
                    ty = xpool.tile([1, COLS], F32, tag="ty")
                for fc in range(4):
                    csl = slice(fc * 400, (fc + 1) * 400)
                    pf = psm.tile([128, 400], F32, tag="psm")
                    nc.tensor.matmul(pf[:], fn1avT[:], AV[:, csl], start=True, stop=False)
                    nc.tensor.matmul(pf[:], fn1xT[:], xt[:, csl], start=False, stop=True)
                    y1 = wpool.tile([128, 400], F32, tag="y1")
                    nc.scalar.activation(y1[:], pf[:], AF.Tanh, bias=fb1[:])
                    pf2 = psm.tile([od, 400], F32, tag="psm2")
                    nc.tensor.matmul(pf2[:], fn2T[:], y1[:], start=True, stop=True)
                    if blk < 2:
                        nc.scalar.activation(xnext[0:od, csl], pf2[:], AF.Tanh, bias=fb2[:])
                    else:
                        nc.scalar.activation(ty[:, csl], pf2[:], AF.Tanh, bias=fb2[:])
                if blk < 2:
                    nc.sync.dma_start(xnext[od:od + 2, :], xt0[:, :])
                    xt = xnext

            # ---------- final: sigmoid(mean over N) ----------
            red = xpool.tile([1, GPC], F32, tag="red")
            nc.vector.tensor_reduce(red[:], ty[:].rearrange("p (a b) -> p a b", a=GPC),
                                    axis=AX.X, op=ALU.add)
            osb = xpool.tile([1, GPC], F32, tag="osb")
            nc.scalar.activation(osb[:], red[:], AF.Sigmoid, scale=1.0 / N)
            nc.sync.dma_start(out_d.ap(), osb[:])

    nc.compile()
    return nc


def _host_prep(inputs):
    """Build per-core in_maps from full inputs."""
    x = np.asarray(inputs["x"], np.float32)          # [B, N, 2]
    shared = {}
    for i in range(3):
        d = NODE_SIZES[i]
        fe1w = np.asarray(inputs[f"fe1w{i}"], np.float32)   # [128, 2d+1]
        fe1b = np.asarray(inputs[f"fe1b{i}"], np.float32)
        fe2w = np.asarray(inputs[f"fe2w{i}"], np.float32)   # [128, 128]
        fe2b = np.asarray(inputs[f"fe2b{i}"], np.float32)
        fn1w = np.asarray(inputs[f"fn1w{i}"], np.float32)   # [128, 128+d]
        fn1b = np.asarray(inputs[f"fn1b{i}"], np.float32)
        fn2w = np.asarray(inputs[f"fn2w{i}"], np.float32)   # [od, 128]
        fn2b = np.asarray(inputs[f"fn2b{i}"], np.float32)
        if i == 0:
            perm = np.arange(d)
        else:
            # my x row order [y..., c0, c1] -> ref order [c0, c1, y...]
            perm = np.concatenate([np.arange(2, d), [0, 1]])
        W1a = fe1w[:, 0:d][:, perm]
        W1b = fe1w[:, d:2 * d][:, perm]
        w1c = fe1w[:, 2 * d]
        shared[f"w1a{i}"] = round_fp32r(np.ascontiguousarray(W1a.T))
        shared[f"w1b{i}"] = round_fp32r(np.ascontiguousarray(W1b.T))
        shared[f"w1c{i}"] = round_fp32r(w1c.reshape(1, 128))
        shared[f"fe2T{i}"] = round_fp32r(np.ascontiguousarray(fe2w.T))
        shared[f"fn1avT{i}"] = np.ascontiguousarray(fn1w[:, :128].T)
        shared[f"fn1xT{i}"] = round_fp32r(np.ascontiguousarray(fn1w[:, 128:][:, perm].T))
        shared[f"fn2T{i}"] = np.ascontiguousarray(fn2w.T)
        shared[f"b1_{i}"] = fe1b.reshape(128, 1)
        shared[f"b2_{i}"] = fe2b.reshape(128, 1)
        shared[f"fb1_{i}"] = fn1b.reshape(128, 1)
        shared[f"fb2_{i}"] = fn2b.reshape(FN2_OUT[i], 1)

    in_maps = []
    for c in range(NCORES):
        xs = round_fp32r(x[c * GPC:(c + 1) * GPC])           # [16, 100, 2] rounded
        xt0 = np.ascontiguousarray(xs.transpose(2, 0, 1).reshape(2, COLS))
        sq = (xt0[0] * xt0[0] + xt0[1] * xt0[1]).reshape(1, COLS)
        m = dict(shared)
        m["xt0"] = xt0
        m["m2xt"] = -2.0 * xt0
        m["sqv"] = round_fp32r(sq)
        m["onesv"] = np.ones((1, COLS), np.float32)
        in_maps.append(m)
    return in_maps


def kernel(**inputs):
    from concourse import bass_utils

    if "nc" not in _CACHE:
        _CACHE["nc"] = _build()
    nc = _CACHE["nc"]
    in_maps = _host_prep(inputs)
    res = bass_utils.run_bass_kernel_spmd(nc, in_maps, core_ids=list(range(NCORES)))
    out = np.concatenate(
        [np.asarray(res.results[c]["out"], np.float32).reshape(GPC, 1) for c in range(NCORES)],
        axis=0,
    )
    return out


# revision 8
# speedup vs baseline: 545.0162x; 545.0162x over previous
import sys
import numpy as np

for _p in ("/opt/trn_rl_repo", "/root/.axon_site/_ro/trn_rl_repo"):
    if _p not in sys.path:
        sys.path.append(_p)

B, N, NODE, FE = 128, 100, 2, 128
NODE_SIZES = [2, 16, 32]
FN2_OUT = [14, 30, 1]
NCORES = 8
GPC = B // NCORES            # graphs per core = 16
COLS = GPC * N               # 1600
PAIRS = N * N                # 10000
CH = 500                     # matmul moving chunk (<=512 fp32)
IPG = [5, 5, 5, 5, 5, 5, 5, 5, 5, 5, 5, 5, 5, 5, 5, 5, 5, 5, 5, 5]  # i's per chunk


def round_fp32r(a):
    u = np.ascontiguousarray(np.asarray(a, np.float32)).view(np.uint32)
    low = u & np.uint32(0xFFF)
    base = u & np.uint32(0xFFFFF000)
    add = ((low > 0x800) | ((low == 0x800) & (((u >> 12) & 1) == 1))).astype(np.uint32) << 12
    return (base + add).view(np.float32)


_CACHE = {}


def _build():
    import concourse.bacc as bacc
    import concourse.mybir as mybir
    import concourse.tile as tile

    F32 = mybir.dt.float32
    F32R = mybir.dt.float32r
    AF = mybir.ActivationFunctionType
    ALU = mybir.AluOpType
    AX = mybir.AxisListType

    nc = bacc.Bacc("TRN2", target_bir_lowering=False, debug=False, num_devices=NCORES)

    din = {}

    def I(name, shape, dt=F32R):
        din[name] = nc.dram_tensor(name, shape, dt, kind="ExternalInput")

    I("xt0", [2, COLS])
    I("ones10k", [1, PAIRS])
    I("m2xt", [2, COLS])
    I("sqv", [1, COLS])
    I("onesv", [1, COLS])
    for i in range(3):
        d = NODE_SIZES[i]
        od = FN2_OUT[i]
        I(f"w1a{i}", [d, 128])
        I(f"w1b{i}", [d, 128])
        I(f"w1c{i}", [1, 128])
        I(f"w1cb{i}", [2, 128])
        I(f"fe2T{i}", [128, 128])
        I(f"fn1avT{i}", [128, 128], F32)
        I(f"fn1xT{i}", [d, 128])
        I(f"fn2T{i}", [128, od], F32)
        I(f"b1_{i}", [128, 1], F32)
        I(f"b2_{i}", [128, 1], F32)
        I(f"fb1_{i}", [128, 1], F32)
        I(f"fb2_{i}", [od, 1], F32)
    out_d = nc.dram_tensor("out", [1, GPC], F32, kind="ExternalOutput")
    nrm_dram = nc.dram_tensor("nrm_dram", [GPC, PAIRS], F32R)

    # chunk grouping: groups of 2 chunks (1000 cols) -> PSUM tile 2 banks
    GRP_CH = [(c, 2) for c in range(0, 20, 2)]

    with tile.TileContext(nc) as tc:
        with (
            tc.tile_pool(name="const", bufs=1) as cpool,
            tc.tile_pool(name="xp", bufs=1) as xpool,
            tc.tile_pool(name="wk", bufs=3) as wpool,
            tc.tile_pool(name="wk2", bufs=3) as wpool2,
            tc.tile_pool(name="stg", bufs=2) as spool,
            tc.tile_pool(name="ps1", bufs=2, space="PSUM") as ps1,
            tc.tile_pool(name="ps2", bufs=2, space="PSUM") as ps2,
        ):
            W = {}
            for name, dt_ in din.items():
                if name == "ones10k":
                    continue
                sh = list(dt_.shape)
                W[name] = cpool.tile(sh, dt_.dtype, tag=name, name=f"w_{name}")
                nc.sync.dma_start(W[name][:], dt_.ap())

            xt0 = W["xt0"]

            # ---------- phase 0: per-graph pair norms -> nrm_dram ----------
            for g in range(GPC):
                sl = slice(g * N, (g + 1) * N)
                pn = ps1.tile([100, 100], F32, tag="p1")
                nc.tensor.matmul(pn[:], xt0[:, sl], W["m2xt"][:, sl], start=True, stop=False)
                nc.tensor.matmul(pn[:], W["sqv"][:, sl], W["onesv"][:, sl], start=False, stop=False)
                nc.tensor.matmul(pn[:], W["onesv"][:, sl], W["sqv"][:, sl], start=False, stop=True)
                ncl = wpool.tile([100, 100], F32, tag="nclamp")
                nc.vector.tensor_scalar(ncl[:], pn[:], 0.0, None, ALU.max)
                nsq = wpool.tile([100, 100], F32R, tag="nsq")
                nc.scalar.activation(nsq[:], ncl[:], AF.Sqrt)
                nc.sync.dma_start(
                    nrm_dram[g:g + 1, :].rearrange("p (a b) -> p a b", a=100), nsq[:, :]
                )

            # ---------- blocks ----------
            xt = xt0
            ty = None
            for blk in range(3):
                d = NODE_SIZES[blk]
                od = FN2_OUT[blk]
                w1a, w1b, w1c = W[f"w1a{blk}"], W[f"w1b{blk}"], W[f"w1c{blk}"]
                fe2T = W[f"fe2T{blk}"]
                b1, b2 = W[f"b1_{blk}"], W[f"b2_{blk}"]
                AV = xpool.tile([128, COLS], F32, tag=f"av{blk}")

                for g in range(GPC):
                    gsl = slice(g * N, (g + 1) * N)
                    stg = spool.tile([1, PAIRS], F32R, tag="stg")
                    nc.sync.dma_start(stg[0:1, :], nrm_dram[g:g + 1, :])

                    for (c0, nch) in GRP_CH:
                        span = nch * CH
                        p1 = ps1.tile([128, 1024], F32, tag="p1")
                        # fe1 = x1 part + x2 part + norm part (accumulate in PSUM)
                        # chunk ci lives at bank-aligned offset ci*512, cols [0,500)
                        for ci in range(nch):
                            i0 = (c0 + ci) * 5
                            r1 = xt[:, g * N + i0: g * N + i0 + 5].unsqueeze(2).broadcast_to([d, 5, N])
                            nc.tensor.matmul(p1[:, ci * 512: ci * 512 + CH], w1a[:], r1,
                                             start=True, stop=False)
                        for ci in range(nch):
                            r2 = xt[:, gsl].unsqueeze(1).broadcast_to([d, 5, N])
                            nc.tensor.matmul(p1[:, ci * 512: ci * 512 + CH], w1b[:], r2,
                                             start=False, stop=False)
                        on_dve = False
                        for ci in range(nch):
                            cc = (c0 + ci) * CH
                            if on_dve:
                                nc.tensor.matmul(p1[:, ci * 512: ci * 512 + CH],
                                                 W[f"w1cb{blk}"][:], stg[:, cc:cc + CH],
                                                 start=False, stop=True)
                            else:
                                nc.tensor.matmul(p1[:, ci * 512: ci * 512 + CH], w1c[:],
                                                 stg[0:1, cc:cc + CH], start=False, stop=True)
                        # Lrelu (exact, alpha=0.2) -> t1
                        p1v = p1[:, 0:nch * 512].rearrange("p (a b) -> p a b", b=512)[:, :, 0:CH]
                        t1 = wpool.tile([128, 1000], F32R, tag="t1")
                        t1v = t1[:, 0:span].rearrange("p (a b) -> p a b", b=CH)
                        if on_dve:
                            # psum already holds w = z + b1; lrelu = max(0.2w, w)
                            u = wpool.tile([128, 1000], F32, tag="u_dve")
                            uv = u[:, 0:span].rearrange("p (a b) -> p a b", b=CH)
                            nc.vector.tensor_scalar(uv, p1v, 0.2, None, ALU.mult)
                            nc.vector.tensor_tensor(t1v, uv, p1v, ALU.max)
                        else:
                            nc.scalar.activation(t1v, p1v, AF.Prelu,
                                                 bias=b1[:], scale=1.0, alpha=0.2)
                        # fe2
                        p2 = ps2.tile([128, 1024], F32, tag="p2")
                        for ci in range(nch):
                            nc.tensor.matmul(p2[:, ci * 512: ci * 512 + CH], fe2T[:],
                                             t1[:, ci * CH:(ci + 1) * CH], start=True, stop=True)
                        p2v = p2[:, 0:nch * 512].rearrange("p (a b) -> p a b", b=512)[:, :, 0:CH]
                        av2 = wpool2.tile([128, 1000], F32, tag="av2")
                        av2v = av2[:, 0:span].rearrange("p (a b) -> p a b", b=CH)
                        nc.scalar.activation(av2v, p2v, AF.Prelu,
                                             bias=b2[:], scale=1.0, alpha=0.2)
                        # sum over j within each i (segments of 100)
                        n_i = span // N
                        nc.vector.tensor_reduce(
                            AV[:, g * N + c0 * 5: g * N + c0 * 5 + n_i],
                            av2[:, 0:span].rearrange("p (a b) -> p a b", a=n_i),
                            axis=AX.X, op=ALU.add,
                        )

                # ----- fn stage over all COLS -----
                fn1avT, fn1xT = W[f"fn1avT{blk}"], W[f"fn1xT{blk}"]
                fn2T = W[f"fn2T{blk}"]
                fb1, fb2 = W[f"fb1_{blk}"], W[f"fb2_{blk}"]
                if blk < 2:
                    nd = NODE_SIZES[blk + 1]
                    xnext = xpool.tile([nd, COLS], F32R, tag=f"x{blk + 1}")
                else:
                    ty = xpool.tile([1, COLS], F32, tag="ty")
                for fc in range(4):
                    csl = slice(fc * 400, (fc + 1) * 400)
                    pf = ps1.tile([128, 400], F32, tag="p1")
                    nc.tensor.matmul(pf[:], fn1avT[:], AV[:, csl], start=True, stop=False)
                    nc.tensor.matmul(pf[:], fn1xT[:], xt[:, csl], start=False, stop=True)
                    y1 = wpool.tile([128, 400], F32, tag="y1")
                    nc.scalar.activation(y1[:], pf[:], AF.Tanh, bias=fb1[:])
                    pf2 = ps2.tile([od, 400], F32, tag="p2")
                    nc.tensor.matmul(pf2[:], fn2T[:], y1[:], start=True, stop=True)
                    if blk < 2:
                        nc.scalar.activation(xnext[0:od, csl], pf2[:], AF.Tanh, bias=fb2[:])
                    else:
                        nc.scalar.activation(ty[:, csl], pf2[:], AF.Tanh, bias=fb2[:])
                if blk < 2:
                    nc.sync.dma_start(xnext[od:od + 2, :], xt0[:, :])
                    xt = xnext

            # ---------- final: sigmoid(mean over N) ----------
            red = xpool.tile([1, GPC], F32, tag="red")
            nc.vector.tensor_reduce(red[:], ty[:].rearrange("p (a b) -> p a b", a=GPC),
                                    axis=AX.X, op=ALU.add)
            osb = xpool.tile([1, GPC], F32, tag="osb")
            nc.scalar.activation(osb[:], red[:], AF.Sigmoid, scale=1.0 / N)
            nc.sync.dma_start(out_d.ap(), osb[:])

    nc.compile()
    return nc


def _host_prep(inputs):
    """Build per-core in_maps from full inputs."""
    x = np.asarray(inputs["x"], np.float32)          # [B, N, 2]
    shared = {}
    for i in range(3):
        d = NODE_SIZES[i]
        fe1w = np.asarray(inputs[f"fe1w{i}"], np.float32)   # [128, 2d+1]
        fe1b = np.asarray(inputs[f"fe1b{i}"], np.float32)
        fe2w = np.asarray(inputs[f"fe2w{i}"], np.float32)   # [128, 128]
        fe2b = np.asarray(inputs[f"fe2b{i}"], np.float32)
        fn1w = np.asarray(inputs[f"fn1w{i}"], np.float32)   # [128, 128+d]
        fn1b = np.asarray(inputs[f"fn1b{i}"], np.float32)
        fn2w = np.asarray(inputs[f"fn2w{i}"], np.float32)   # [od, 128]
        fn2b = np.asarray(inputs[f"fn2b{i}"], np.float32)
        if i == 0:
            perm = np.arange(d)
        else:
            # my x row order [y..., c0, c1] -> ref order [c0, c1, y...]
            perm = np.concatenate([np.arange(2, d), [0, 1]])
        W1a = fe1w[:, 0:d][:, perm]
        W1b = fe1w[:, d:2 * d][:, perm]
        w1c = fe1w[:, 2 * d]
        shared[f"w1a{i}"] = round_fp32r(np.ascontiguousarray(W1a.T))
        shared[f"w1b{i}"] = round_fp32r(np.ascontiguousarray(W1b.T))
        shared[f"w1c{i}"] = round_fp32r(w1c.reshape(1, 128))
        shared[f"w1cb{i}"] = round_fp32r(np.stack([w1c, fe1b]))
        shared[f"fe2T{i}"] = round_fp32r(np.ascontiguousarray(fe2w.T))
        shared[f"fn1avT{i}"] = np.ascontiguousarray(fn1w[:, :128].T)
        shared[f"fn1xT{i}"] = round_fp32r(np.ascontiguousarray(fn1w[:, 128:][:, perm].T))
        shared[f"fn2T{i}"] = np.ascontiguousarray(fn2w.T)
        shared[f"b1_{i}"] = fe1b.reshape(128, 1)
        shared[f"b2_{i}"] = fe2b.reshape(128, 1)
        shared[f"fb1_{i}"] = fn1b.reshape(128, 1)
        shared[f"fb2_{i}"] = fn2b.reshape(FN2_OUT[i], 1)

    in_maps = []
    for c in range(NCORES):
        xs = round_fp32r(x[c * GPC:(c + 1) * GPC])           # [16, 100, 2] rounded
        xt0 = np.ascontiguousarray(xs.transpose(2, 0, 1).reshape(2, COLS))
        sq = (xt0[0] * xt0[0] + xt0[1] * xt0[1]).reshape(1, COLS)
        m = dict(shared)
        m["xt0"] = xt0
        m["m2xt"] = -2.0 * xt0
        m["sqv"] = round_fp32r(sq)
        m["onesv"] = np.ones((1, COLS), np.float32)
        m["ones10k"] = np.ones((1, PAIRS), np.float32)
        in_maps.append(m)
    return in_maps


def kernel(**inputs):
    from concourse import bass_utils

    if "nc" not in _CACHE:
        _CACHE["nc"] = _build()
    nc = _CACHE["nc"]
    in_maps = _host_prep(inputs)
    res = bass_utils.run_bass_kernel_spmd(nc, in_maps, core_ids=list(range(NCORES)))
    out = np.concatenate(
        [np.asarray(res.results[c]["out"], np.float32).reshape(GPC, 1) for c in range(NCORES)],
        axis=0,
    )
    return out


# revision 9
# speedup vs baseline: 547.0096x; 1.0037x over previous
import sys
import numpy as np

for _p in ("/opt/trn_rl_repo", "/root/.axon_site/_ro/trn_rl_repo"):
    if _p not in sys.path:
        sys.path.append(_p)

B, N, NODE, FE = 128, 100, 2, 128
NODE_SIZES = [2, 16, 32]
FN2_OUT = [14, 30, 1]
NCORES = 8
GPC = B // NCORES            # graphs per core = 16
COLS = GPC * N               # 1600
PAIRS = N * N                # 10000
CH = 500                     # matmul moving chunk (<=512 fp32)
IPG = [5, 5, 5, 5, 5, 5, 5, 5, 5, 5, 5, 5, 5, 5, 5, 5, 5, 5, 5, 5]  # i's per chunk


def round_fp32r(a):
    u = np.ascontiguousarray(np.asarray(a, np.float32)).view(np.uint32)
    low = u & np.uint32(0xFFF)
    base = u & np.uint32(0xFFFFF000)
    add = ((low > 0x800) | ((low == 0x800) & (((u >> 12) & 1) == 1))).astype(np.uint32) << 12
    return (base + add).view(np.float32)


_CACHE = {}


def _build():
    import concourse.bacc as bacc
    import concourse.mybir as mybir
    import concourse.tile as tile

    F32 = mybir.dt.float32
    F32R = mybir.dt.float32r
    AF = mybir.ActivationFunctionType
    ALU = mybir.AluOpType
    AX = mybir.AxisListType

    nc = bacc.Bacc("TRN2", target_bir_lowering=False, debug=False, num_devices=NCORES)

    din = {}

    def I(name, shape, dt=F32R):
        din[name] = nc.dram_tensor(name, shape, dt, kind="ExternalInput")

    I("xt0", [2, COLS])
    I("ones10k", [1, PAIRS])
    I("m2xt", [2, COLS])
    I("sqv", [1, COLS])
    I("onesv", [1, COLS])
    for i in range(3):
        d = NODE_SIZES[i]
        od = FN2_OUT[i]
        I(f"w1a{i}", [d, 128])
        I(f"w1b{i}", [d, 128])
        I(f"w1c{i}", [1, 128])
        I(f"w1cb{i}", [2, 128])
        I(f"fe2T{i}", [128, 128])
        I(f"fn1avT{i}", [128, 128], F32)
        I(f"fn1xT{i}", [d, 128])
        I(f"fn2T{i}", [128, od], F32)
        I(f"b1_{i}", [128, 1], F32)
        I(f"b2_{i}", [128, 1], F32)
        I(f"fb1_{i}", [128, 1], F32)
        I(f"fb2_{i}", [od, 1], F32)
    out_d = nc.dram_tensor("out", [1, GPC], F32, kind="ExternalOutput")
    nrm_dram = nc.dram_tensor("nrm_dram", [GPC, PAIRS], F32R)

    # chunk grouping: groups of 2 chunks (1000 cols) -> PSUM tile 2 banks
    GRP_CH = [(c, 2) for c in range(0, 20, 2)]

    with tile.TileContext(nc) as tc:
        with (
            tc.tile_pool(name="const", bufs=1) as cpool,
            tc.tile_pool(name="xp", bufs=1) as xpool,
            tc.tile_pool(name="wk", bufs=4) as wpool,
            tc.tile_pool(name="wk2", bufs=4) as wpool2,
            tc.tile_pool(name="stg", bufs=2) as spool,
            tc.tile_pool(name="ps1", bufs=2, space="PSUM") as ps1,
            tc.tile_pool(name="ps2", bufs=2, space="PSUM") as ps2,
        ):
            W = {}
            for name, dt_ in din.items():
                if name == "ones10k":
                    continue
                sh = list(dt_.shape)
                W[name] = cpool.tile(sh, dt_.dtype, tag=name, name=f"w_{name}")
                nc.sync.dma_start(W[name][:], dt_.ap())

            xt0 = W["xt0"]

            # ---------- phase 0: per-graph pair norms -> nrm_dram ----------
            for g in range(GPC):
                sl = slice(g * N, (g + 1) * N)
                pn = ps1.tile([100, 100], F32, tag="p1")
                nc.tensor.matmul(pn[:], xt0[:, sl], W["m2xt"][:, sl], start=True, stop=False)
                nc.tensor.matmul(pn[:], W["sqv"][:, sl], W["onesv"][:, sl], start=False, stop=False)
                nc.tensor.matmul(pn[:], W["onesv"][:, sl], W["sqv"][:, sl], start=False, stop=True)
                ncl = wpool.tile([100, 100], F32, tag="nclamp")
                nc.vector.tensor_scalar(ncl[:], pn[:], 0.0, None, ALU.max)
                nsq = wpool.tile([100, 100], F32R, tag="nsq")
                nc.scalar.activation(nsq[:], ncl[:], AF.Sqrt)
                nc.sync.dma_start(
                    nrm_dram[g:g + 1, :].rearrange("p (a b) -> p a b", a=100), nsq[:, :]
                )

            # ---------- blocks ----------
            xt = xt0
            ty = None
            for blk in range(3):
                d = NODE_SIZES[blk]
                od = FN2_OUT[blk]
                w1a, w1b, w1c = W[f"w1a{blk}"], W[f"w1b{blk}"], W[f"w1c{blk}"]
                fe2T = W[f"fe2T{blk}"]
                b1, b2 = W[f"b1_{blk}"], W[f"b2_{blk}"]
                AV = xpool.tile([128, COLS], F32, tag=f"av{blk}")

                for g in range(GPC):
                    gsl = slice(g * N, (g + 1) * N)
                    stg = spool.tile([1, PAIRS], F32R, tag="stg")
                    nc.sync.dma_start(stg[0:1, :], nrm_dram[g:g + 1, :])

                    for (c0, nch) in GRP_CH:
                        span = nch * CH
                        p1 = ps1.tile([128, 1024], F32, tag="p1")
                        # fe1 = x1 part + x2 part + norm part (accumulate in PSUM)
                        # chunk ci lives at bank-aligned offset ci*512, cols [0,500)
                        for ci in range(nch):
                            i0 = (c0 + ci) * 5
                            r1 = xt[:, g * N + i0: g * N + i0 + 5].unsqueeze(2).broadcast_to([d, 5, N])
                            nc.tensor.matmul(p1[:, ci * 512: ci * 512 + CH], w1a[:], r1,
                                             start=True, stop=False)
                        for ci in range(nch):
                            r2 = xt[:, gsl].unsqueeze(1).broadcast_to([d, 5, N])
                            nc.tensor.matmul(p1[:, ci * 512: ci * 512 + CH], w1b[:], r2,
                                             start=False, stop=False)
                        on_dve = False
                        for ci in range(nch):
                            cc = (c0 + ci) * CH
                            if on_dve:
                                nc.tensor.matmul(p1[:, ci * 512: ci * 512 + CH],
                                                 W[f"w1cb{blk}"][:], stg[:, cc:cc + CH],
                                                 start=False, stop=True)
                            else:
                                nc.tensor.matmul(p1[:, ci * 512: ci * 512 + CH], w1c[:],
                                                 stg[0:1, cc:cc + CH], start=False, stop=True)
                        # Lrelu (exact, alpha=0.2) -> t1
                        p1v = p1[:, 0:nch * 512].rearrange("p (a b) -> p a b", b=512)[:, :, 0:CH]
                        t1 = wpool.tile([128, 1000], F32R, tag="t1")
                        t1v = t1[:, 0:span].rearrange("p (a b) -> p a b", b=CH)
                        if on_dve:
                            # psum already holds w = z + b1; lrelu = max(0.2w, w)
                            u = wpool.tile([128, 1000], F32, tag="u_dve")
                            uv = u[:, 0:span].rearrange("p (a b) -> p a b", b=CH)
                            nc.vector.tensor_scalar(uv, p1v, 0.2, None, ALU.mult)
                            nc.vector.tensor_tensor(t1v, uv, p1v, ALU.max)
                        else:
                            nc.scalar.activation(t1v, p1v, AF.Prelu,
                                                 bias=b1[:], scale=1.0, alpha=0.2)
                        # fe2
                        p2 = ps2.tile([128, 1024], F32, tag="p2")
                        for ci in range(nch):
                            nc.tensor.matmul(p2[:, ci * 512: ci * 512 + CH], fe2T[:],
                                             t1[:, ci * CH:(ci + 1) * CH], start=True, stop=True)
                        p2v = p2[:, 0:nch * 512].rearrange("p (a b) -> p a b", b=512)[:, :, 0:CH]
                        av2 = wpool2.tile([128, 1000], F32, tag="av2")
                        av2v = av2[:, 0:span].rearrange("p (a b) -> p a b", b=CH)
                        nc.scalar.activation(av2v, p2v, AF.Prelu,
                                             bias=b2[:], scale=1.0, alpha=0.2)
                        # sum over j within each i (segments of 100)
                        n_i = span // N
                        nc.vector.tensor_reduce(
                            AV[:, g * N + c0 * 5: g * N + c0 * 5 + n_i],
                            av2[:, 0:span].rearrange("p (a b) -> p a b", a=n_i),
                            axis=AX.X, op=ALU.add,
                        )

                # ----- fn stage over all COLS -----
                fn1avT, fn1xT = W[f"fn1avT{blk}"], W[f"fn1xT{blk}"]
                fn2T = W[f"fn2T{blk}"]
                fb1, fb2 = W[f"fb1_{blk}"], W[f"fb2_{blk}"]
                if blk < 2:
                    nd = NODE_SIZES[blk + 1]
                    xnext = xpool.tile([nd, COLS], F32R, tag=f"x{blk + 1}")
                else:
                    ty = xpool.tile([1, COLS], F32, tag="ty")
                for fc in range(4):
                    csl = slice(fc * 400, (fc + 1) * 400)
                    pf = ps1.tile([128, 400], F32, tag="p1")
                    nc.tensor.matmul(pf[:], fn1avT[:], AV[:, csl], start=True, stop=False)
                    nc.tensor.matmul(pf[:], fn1xT[:], xt[:, csl], start=False, stop=True)
                    y1 = wpool.tile([128, 400], F32, tag="y1")
                    nc.scalar.activation(y1[:], pf[:], AF.Tanh, bias=fb1[:])
                    pf2 = ps2.tile([od, 400], F32, tag="p2")
                    nc.tensor.matmul(pf2[:], fn2T[:], y1[:], start=True, stop=True)
                    if blk < 2:
                        nc.scalar.activation(xnext[0:od, csl], pf2[:], AF.Tanh, bias=fb2[:])
                    else:
                        nc.scalar.activation(ty[:, csl], pf2[:], AF.Tanh, bias=fb2[:])
                if blk < 2:
                    nc.sync.dma_start(xnext[od:od + 2, :], xt0[:, :])
                    xt = xnext

            # ---------- final: sigmoid(mean over N) ----------
            red = xpool.tile([1, GPC], F32, tag="red")
            nc.vector.tensor_reduce(red[:], ty[:].rearrange("p (a b) -> p a b", a=GPC),
                                    axis=AX.X, op=ALU.add)
            osb = xpool.tile([1, GPC], F32, tag="osb")
            nc.scalar.activation(osb[:], red[:], AF.Sigmoid, scale=1.0 / N)
            nc.sync.dma_start(out_d.ap(), osb[:])

    nc.compile()
    return nc


def _host_prep(inputs):
    """Build per-core in_maps from full inputs."""
    x = np.asarray(inputs["x"], np.float32)          # [B, N, 2]
    shared = {}
    for i in range(3):
        d = NODE_SIZES[i]
        fe1w = np.asarray(inputs[f"fe1w{i}"], np.float32)   # [128, 2d+1]
        fe1b = np.asarray(inputs[f"fe1b{i}"], np.float32)
        fe2w = np.asarray(inputs[f"fe2w{i}"], np.float32)   # [128, 128]
        fe2b = np.asarray(inputs[f"fe2b{i}"], np.float32)
        fn1w = np.asarray(inputs[f"fn1w{i}"], np.float32)   # [128, 128+d]
        fn1b = np.asarray(inputs[f"fn1b{i}"], np.float32)
        fn2w = np.asarray(inputs[f"fn2w{i}"], np.float32)   # [od, 128]
        fn2b = np.asarray(inputs[f"fn2b{i}"], np.float32)
        if i == 0:
            perm = np.arange(d)
        else:
            # my x row order [y..., c0, c1] -> ref order [c0, c1, y...]
            perm = np.concatenate([np.arange(2, d), [0, 1]])
        W1a = fe1w[:, 0:d][:, perm]
        W1b = fe1w[:, d:2 * d][:, perm]
        w1c = fe1w[:, 2 * d]
        shared[f"w1a{i}"] = round_fp32r(np.ascontiguousarray(W1a.T))
        shared[f"w1b{i}"] = round_fp32r(np.ascontiguousarray(W1b.T))
        shared[f"w1c{i}"] = round_fp32r(w1c.reshape(1, 128))
        shared[f"w1cb{i}"] = round_fp32r(np.stack([w1c, fe1b]))
        shared[f"fe2T{i}"] = round_fp32r(np.ascontiguousarray(fe2w.T))
        shared[f"fn1avT{i}"] = np.ascontiguousarray(fn1w[:, :128].T)
        shared[f"fn1xT{i}"] = round_fp32r(np.ascontiguousarray(fn1w[:, 128:][:, perm].T))
        shared[f"fn2T{i}"] = np.ascontiguousarray(fn2w.T)
        shared[f"b1_{i}"] = fe1b.reshape(128, 1)
        shared[f"b2_{i}"] = fe2b.reshape(128, 1)
        shared[f"fb1_{i}"] = fn1b.reshape(128, 1)
        shared[f"fb2_{i}"] = fn2b.reshape(FN2_OUT[i], 1)

    in_maps = []
    for c in range(NCORES):
        xs = round_fp32r(x[c * GPC:(c + 1) * GPC])           # [16, 100, 2] rounded
        xt0 = np.ascontiguousarray(xs.transpose(2, 0, 1).reshape(2, COLS))
        sq = (xt0[0] * xt0[0] + xt0[1] * xt0[1]).reshape(1, COLS)
        m = dict(shared)
        m["xt0"] = xt0
        m["m2xt"] = -2.0 * xt0
        m["sqv"] = round_fp32r(sq)
        m["onesv"] = np.ones((1, COLS), np.float32)
        m["ones10k"] = np.ones((1, PAIRS), np.float32)
        in_maps.append(m)
    return in_maps


def kernel(**inputs):
    from concourse import bass_utils

    if "nc" not in _CACHE:
        _CACHE["nc"] = _build()
    nc = _CACHE["nc"]
    in_maps = _host_prep(inputs)
    res = bass_utils.run_bass_kernel_spmd(nc, in_maps, core_ids=list(range(NCORES)))
    out = np.concatenate(
        [np.asarray(res.results[c]["out"], np.float32).reshape(GPC, 1) for c in range(NCORES)],
        axis=0,
    )
    return out


# revision 10
# speedup vs baseline: 548.6397x; 1.0030x over previous
import sys
import numpy as np

for _p in ("/opt/trn_rl_repo", "/root/.axon_site/_ro/trn_rl_repo"):
    if _p not in sys.path:
        sys.path.append(_p)

B, N, NODE, FE = 128, 100, 2, 128
NODE_SIZES = [2, 16, 32]
FN2_OUT = [14, 30, 1]
NCORES = 8
GPC = B // NCORES            # graphs per core = 16
COLS = GPC * N               # 1600
PAIRS = N * N                # 10000
CH = 500                     # matmul moving chunk (<=512 fp32)
IPG = [5, 5, 5, 5, 5, 5, 5, 5, 5, 5, 5, 5, 5, 5, 5, 5, 5, 5, 5, 5]  # i's per chunk


def round_fp32r(a):
    u = np.ascontiguousarray(np.asarray(a, np.float32)).view(np.uint32)
    low = u & np.uint32(0xFFF)
    base = u & np.uint32(0xFFFFF000)
    add = ((low > 0x800) | ((low == 0x800) & (((u >> 12) & 1) == 1))).astype(np.uint32) << 12
    return (base + add).view(np.float32)


_CACHE = {}


def _build():
    import concourse.bacc as bacc
    import concourse.mybir as mybir
    import concourse.tile as tile

    F32 = mybir.dt.float32
    F32R = mybir.dt.float32r
    AF = mybir.ActivationFunctionType
    ALU = mybir.AluOpType
    AX = mybir.AxisListType

    nc = bacc.Bacc("TRN2", target_bir_lowering=False, debug=False, num_devices=NCORES)

    din = {}

    def I(name, shape, dt=F32R):
        din[name] = nc.dram_tensor(name, shape, dt, kind="ExternalInput")

    I("xt0", [2, COLS])
    I("ones10k", [1, PAIRS])
    I("m2xt", [2, COLS])
    I("sqv", [1, COLS])
    I("onesv", [1, COLS])
    for i in range(3):
        d = NODE_SIZES[i]
        od = FN2_OUT[i]
        I(f"w1a{i}", [d, 128])
        I(f"w1b{i}", [d, 128])
        I(f"w1c{i}", [1, 128])
        I(f"w1cb{i}", [2, 128])
        I(f"fe2T{i}", [128, 128])
        I(f"fn1avT{i}", [128, 128], F32)
        I(f"fn1xT{i}", [d, 128])
        I(f"fn2T{i}", [128, od], F32)
        I(f"b1_{i}", [128, 1], F32)
        I(f"b2_{i}", [128, 1], F32)
        I(f"fb1_{i}", [128, 1], F32)
        I(f"fb2_{i}", [od, 1], F32)
    out_d = nc.dram_tensor("out", [1, GPC], F32, kind="ExternalOutput")
    nrm_dram = nc.dram_tensor("nrm_dram", [GPC, PAIRS], F32R)

    # chunk grouping: groups of 2 chunks (1000 cols) -> PSUM tile 2 banks
    GRP_CH = [(c, 2) for c in range(0, 20, 2)]

    with tile.TileContext(nc) as tc:
        with (
            tc.tile_pool(name="const", bufs=1) as cpool,
            tc.tile_pool(name="xp", bufs=1) as xpool,
            tc.tile_pool(name="wk", bufs=4) as wpool,
            tc.tile_pool(name="wk2", bufs=4) as wpool2,
            tc.tile_pool(name="stg", bufs=2) as spool,
            tc.tile_pool(name="ps1", bufs=3, space="PSUM") as ps1,
            tc.tile_pool(name="ps2", bufs=1, space="PSUM") as ps2,
        ):
            W = {}
            for name, dt_ in din.items():
                if name == "ones10k":
                    continue
                sh = list(dt_.shape)
                W[name] = cpool.tile(sh, dt_.dtype, tag=name, name=f"w_{name}")
                nc.sync.dma_start(W[name][:], dt_.ap())

            xt0 = W["xt0"]

            # ---------- phase 0: per-graph pair norms -> nrm_dram ----------
            for g in range(GPC):
                sl = slice(g * N, (g + 1) * N)
                pn = ps1.tile([100, 100], F32, tag="p1")
                nc.tensor.matmul(pn[:], xt0[:, sl], W["m2xt"][:, sl], start=True, stop=False)
                nc.tensor.matmul(pn[:], W["sqv"][:, sl], W["onesv"][:, sl], start=False, stop=False)
                nc.tensor.matmul(pn[:], W["onesv"][:, sl], W["sqv"][:, sl], start=False, stop=True)
                ncl = wpool.tile([100, 100], F32, tag="nclamp")
                nc.vector.tensor_scalar(ncl[:], pn[:], 0.0, None, ALU.max)
                nsq = wpool.tile([100, 100], F32R, tag="nsq")
                nc.scalar.activation(nsq[:], ncl[:], AF.Sqrt)
                nc.sync.dma_start(
                    nrm_dram[g:g + 1, :].rearrange("p (a b) -> p a b", a=100), nsq[:, :]
                )

            # ---------- blocks ----------
            xt = xt0
            ty = None
            for blk in range(3):
                d = NODE_SIZES[blk]
                od = FN2_OUT[blk]
                w1a, w1b, w1c = W[f"w1a{blk}"], W[f"w1b{blk}"], W[f"w1c{blk}"]
                fe2T = W[f"fe2T{blk}"]
                b1, b2 = W[f"b1_{blk}"], W[f"b2_{blk}"]
                AV = xpool.tile([128, COLS], F32, tag=f"av{blk}")

                for g in range(GPC):
                    gsl = slice(g * N, (g + 1) * N)
                    stg = spool.tile([1, PAIRS], F32R, tag="stg")
                    nc.sync.dma_start(stg[0:1, :], nrm_dram[g:g + 1, :])

                    for (c0, nch) in GRP_CH:
                        span = nch * CH
                        p1 = ps1.tile([128, 1024], F32, tag="p1")
                        # fe1 = x1 part + x2 part + norm part (accumulate in PSUM)
                        # chunk ci lives at bank-aligned offset ci*512, cols [0,500)
                        for ci in range(nch):
                            i0 = (c0 + ci) * 5
                            r1 = xt[:, g * N + i0: g * N + i0 + 5].unsqueeze(2).broadcast_to([d, 5, N])
                            nc.tensor.matmul(p1[:, ci * 512: ci * 512 + CH], w1a[:], r1,
                                             start=True, stop=False)
                        for ci in range(nch):
                            r2 = xt[:, gsl].unsqueeze(1).broadcast_to([d, 5, N])
                            nc.tensor.matmul(p1[:, ci * 512: ci * 512 + CH], w1b[:], r2,
                                             start=False, stop=False)
                        on_dve = False
                        for ci in range(nch):
                            cc = (c0 + ci) * CH
                            if on_dve:
                                nc.tensor.matmul(p1[:, ci * 512: ci * 512 + CH],
                                                 W[f"w1cb{blk}"][:], stg[:, cc:cc + CH],
                                                 start=False, stop=True)
                            else:
                                nc.tensor.matmul(p1[:, ci * 512: ci * 512 + CH], w1c[:],
                                                 stg[0:1, cc:cc + CH], start=False, stop=True)
                        # Lrelu (exact, alpha=0.2) -> t1
                        p1v = p1[:, 0:nch * 512].rearrange("p (a b) -> p a b", b=512)[:, :, 0:CH]
                        t1 = wpool.tile([128, 1000], F32R, tag="t1")
                        t1v = t1[:, 0:span].rearrange("p (a b) -> p a b", b=CH)
                        if on_dve:
                            # psum already holds w = z + b1; lrelu = max(0.2w, w)
                            u = wpool.tile([128, 1000], F32, tag="u_dve")
                            uv = u[:, 0:span].rearrange("p (a b) -> p a b", b=CH)
                            nc.vector.tensor_scalar(uv, p1v, 0.2, None, ALU.mult)
                            nc.vector.tensor_tensor(t1v, uv, p1v, ALU.max)
                        else:
                            nc.scalar.activation(t1v, p1v, AF.Prelu,
                                                 bias=b1[:], scale=1.0, alpha=0.2)
                        # fe2
                        p2 = ps2.tile([128, 1024], F32, tag="p2")
                        for ci in range(nch):
                            nc.tensor.matmul(p2[:, ci * 512: ci * 512 + CH], fe2T[:],
                                             t1[:, ci * CH:(ci + 1) * CH], start=True, stop=True)
                        p2v = p2[:, 0:nch * 512].rearrange("p (a b) -> p a b", b=512)[:, :, 0:CH]
                        av2 = wpool2.tile([128, 1000], F32, tag="av2")
                        av2v = av2[:, 0:span].rearrange("p (a b) -> p a b", b=CH)
                        nc.scalar.activation(av2v, p2v, AF.Prelu,
                                             bias=b2[:], scale=1.0, alpha=0.2)
                        # sum over j within each i (segments of 100)
                        n_i = span // N
                        nc.vector.tensor_reduce(
                            AV[:, g * N + c0 * 5: g * N + c0 * 5 + n_i],
                            av2[:, 0:span].rearrange("p (a b) -> p a b", a=n_i),
                            axis=AX.X, op=ALU.add,
                        )

                # ----- fn stage over all COLS -----
                fn1avT, fn1xT = W[f"fn1avT{blk}"], W[f"fn1xT{blk}"]
                fn2T = W[f"fn2T{blk}"]
                fb1, fb2 = W[f"fb1_{blk}"], W[f"fb2_{blk}"]
                if blk < 2:
                    nd = NODE_SIZES[blk + 1]
                    xnext = xpool.tile([nd, COLS], F32R, tag=f"x{blk + 1}")
                else:
                    ty = xpool.tile([1, COLS], F32, tag="ty")
                for fc in range(4):
                    csl = slice(fc * 400, (fc + 1) * 400)
                    pf = ps1.tile([128, 400], F32, tag="p1")
                    nc.tensor.matmul(pf[:], fn1avT[:], AV[:, csl], start=True, stop=False)
                    nc.tensor.matmul(pf[:], fn1xT[:], xt[:, csl], start=False, stop=True)
                    y1 = wpool.tile([128, 400], F32, tag="y1")
                    nc.scalar.activation(y1[:], pf[:], AF.Tanh, bias=fb1[:])
                    pf2 = ps2.tile([od, 400], F32, tag="p2")
                    nc.tensor.matmul(pf2[:], fn2T[:], y1[:], start=True, stop=True)
                    if blk < 2:
                        nc.scalar.activation(xnext[0:od, csl], pf2[:], AF.Tanh, bias=fb2[:])
                    else:
                        nc.scalar.activation(ty[:, csl], pf2[:], AF.Tanh, bias=fb2[:])
                if blk < 2:
                    nc.sync.dma_start(xnext[od:od + 2, :], xt0[:, :])
                    xt = xnext

            # ---------- final: sigmoid(mean over N) ----------
            red = xpool.tile([1, GPC], F32, tag="red")
            nc.vector.tensor_reduce(red[:], ty[:].rearrange("p (a b) -> p a b", a=GPC),
                                    axis=AX.X, op=ALU.add)
            osb = xpool.tile([1, GPC], F32, tag="osb")
            nc.scalar.activation(osb[:], red[:], AF.Sigmoid, scale=1.0 / N)
            nc.sync.dma_start(out_d.ap(), osb[:])

    nc.compile()
    return nc


def _host_prep(inputs):
    """Build per-core in_maps from full inputs."""
    x = np.asarray(inputs["x"], np.float32)          # [B, N, 2]
    shared = {}
    for i in range(3):
        d = NODE_SIZES[i]
        fe1w = np.asarray(inputs[f"fe1w{i}"], np.float32)   # [128, 2d+1]
        fe1b = np.asarray(inputs[f"fe1b{i}"], np.float32)
        fe2w = np.asarray(inputs[f"fe2w{i}"], np.float32)   # [128, 128]
        fe2b = np.asarray(inputs[f"fe2b{i}"], np.float32)
        fn1w = np.asarray(inputs[f"fn1w{i}"], np.float32)   # [128, 128+d]
        fn1b = np.asarray(inputs[f"fn1b{i}"], np.float32)
        fn2w = np.asarray(inputs[f"fn2w{i}"], np.float32)   # [od, 128]
        fn2b = np.asarray(inputs[f"fn2b{i}"], np.float32)
        if i == 0:
            perm = np.arange(d)
        else:
            # my x row order [y..., c0, c1] -> ref order [c0, c1, y...]
            perm = np.concatenate([np.arange(2, d), [0, 1]])
        W1a = fe1w[:, 0:d][:, perm]
        W1b = fe1w[:, d:2 * d][:, perm]
        w1c = fe1w[:, 2 * d]
        shared[f"w1a{i}"] = round_fp32r(np.ascontiguousarray(W1a.T))
        shared[f"w1b{i}"] = round_fp32r(np.ascontiguousarray(W1b.T))
        shared[f"w1c{i}"] = round_fp32r(w1c.reshape(1, 128))
        shared[f"w1cb{i}"] = round_fp32r(np.stack([w1c, fe1b]))
        shared[f"fe2T{i}"] = round_fp32r(np.ascontiguousarray(fe2w.T))
        shared[f"fn1avT{i}"] = np.ascontiguousarray(fn1w[:, :128].T)
        shared[f"fn1xT{i}"] = round_fp32r(np.ascontiguousarray(fn1w[:, 128:][:, perm].T))
        shared[f"fn2T{i}"] = np.ascontiguousarray(fn2w.T)
        shared[f"b1_{i}"] = fe1b.reshape(128, 1)
        shared[f"b2_{i}"] = fe2b.reshape(128, 1)
        shared[f"fb1_{i}"] = fn1b.reshape(128, 1)
        shared[f"fb2_{i}"] = fn2b.reshape(FN2_OUT[i], 1)

    in_maps = []
    for c in range(NCORES):
        xs = round_fp32r(x[c * GPC:(c + 1) * GPC])           # [16, 100, 2] rounded
        xt0 = np.ascontiguousarray(xs.transpose(2, 0, 1).reshape(2, COLS))
        sq = (xt0[0] * xt0[0] + xt0[1] * xt0[1]).reshape(1, COLS)
        m = dict(shared)
        m["xt0"] = xt0
        m["m2xt"] = -2.0 * xt0
        m["sqv"] = round_fp32r(sq)
        m["onesv"] = np.ones((1, COLS), np.float32)
        m["ones10k"] = np.ones((1, PAIRS), np.float32)
        in_maps.append(m)
    return in_maps


def kernel(**inputs):
    from concourse import bass_utils

    if "nc" not in _CACHE:
        _CACHE["nc"] = _build()
    nc = _CACHE["nc"]
    in_maps = _host_prep(inputs)
    res = bass_utils.run_bass_kernel_spmd(nc, in_maps, core_ids=list(range(NCORES)))
    out = np.concatenate(
        [np.asarray(res.results[c]["out"], np.float32).reshape(GPC, 1) for c in range(NCORES)],
        axis=0,
    )
    return out
